# revision 21
# baseline (speedup 1.0000x reference)
"""Local (banded) attention -> mean over sequence, on 8 TRN2 NeuronCores.

Math (same reduction as the earlier kernel):
  scores'[i,j] = qa_i . x_j with qa = x @ A + cb, A = Wq Wk^T / sqrt(H),
  cb = Wk bq / sqrt(H); out = (u/S) @ Wv + bv with u = sum_j tw_j x_j,
  tw_j = sum_i exp(s_ij) / rs_i (banded).

Pipeline layout (per core: 2048 queries, NK=2304 keys incl zero-pad halo):
  - PE warms up on dummy matmuls so the p-state ramp (3us to full clock)
    completes while the DMAs fill SBUF.
  - qa projection in 4 column sub-chunks; psum->sbuf bias-copies split
    between DVE and Act, the late ones deferred into the pair loop so no
    in-order queue head-of-line blocks the stream (GPSIMD cannot access
    PSUM, so Pool only gets SBUF-to-SBUF work).
  - scores+exp processed in block PAIRS ([128,768] psum tiles, one Exp).
  - banded mask + row-sum via one DVE scalar_tensor_tensor per block (4x
    perf mode); per-core edge masks also zero the padded-key columns, so
    no row-count correction pass is needed.
  - tw (per-key total weight) via free-size-1 matmuls; u accumulated in
    "tall" form (lhsT = xn chunk, rhs = twT column) - almost free on PE.
  - host epilogue applies (u/S) @ Wv + bv.
Sharding: 8 cores = batch(4) x sequence-half(2), 128-key halo.
"""

import numpy as np
import ml_dtypes

B, S, H = 4, 4096, 256
W = 128          # window size this kernel is specialized for
SH = S // 2      # query rows per core
HALO = 128
NK = SH + 2 * HALO   # keys per core incl. zero-padded halo
NKC = NK // 128      # 18 key chunks
NQB = SH // 128      # 16 query blocks
NPAIR = NQB // 2     # 8 block pairs
MXW = 1152 + NKC * 256   # masks (3x384) + xn chunk-major
BF16 = ml_dtypes.bfloat16

_CACHE = {}
DEBUG_DUMP = False


def _build():
    import concourse.bass as bass
    import concourse.tile as tile
    import concourse.mybir as mybir
    from concourse import bacc

    f32 = mybir.dt.float32
    bf16 = mybir.dt.bfloat16
    AF = mybir.ActivationFunctionType
    ALU = mybir.AluOpType

    nc = bacc.Bacc(
        "TRN2", target_bir_lowering=False, debug=False,
        enable_asserts=False, num_devices=1,
    )

    xt_d = nc.dram_tensor("xt", [256, NK], bf16, kind="ExternalInput").ap()
    ap_d = nc.dram_tensor("apk", [128, 516], bf16, kind="ExternalInput").ap()
    mxn_d = nc.dram_tensor("mxn", [128, MXW], bf16, kind="ExternalInput").ap()
    u_d = nc.dram_tensor("u", [128, 2], f32, kind="ExternalOutput").ap()
    if DEBUG_DUMP:
        dqa0_d = nc.dram_tensor("dqa0", [128, SH], bf16, kind="ExternalOutput").ap()
        dqa1_d = nc.dram_tensor("dqa1", [128, SH], bf16, kind="ExternalOutput").ap()
        drs_d = nc.dram_tensor("drs", [128, NQB], f32, kind="ExternalOutput").ap()
        dtw_d = nc.dram_tensor("dtw", [128, NKC], bf16, kind="ExternalOutput").ap()

    with tile.TileContext(nc) as tc:
        with (
            tc.tile_pool(name="cst", bufs=1) as cst,
            tc.tile_pool(name="wrk", bufs=1) as wrk,
            tc.tile_pool(name="pqa", bufs=3, space="PSUM") as pqa,
            tc.tile_pool(name="ppr", bufs=2, space="PSUM") as ppr,
            tc.tile_pool(name="ptw", bufs=1, space="PSUM") as ptw,
        ):
            xtw = cst.tile([128, 2 * NK], bf16, tag="xtw")
            xtv = xtw.rearrange("p (t k) -> p t k", t=2)
            apk = cst.tile([128, 516], bf16, tag="apk")
            mxn = cst.tile([128, MXW], bf16, tag="mxn")
            qa0 = cst.tile([128, SH], bf16, tag="qa0")
            qa1 = cst.tile([128, SH], bf16, tag="qa1")
            rs = cst.tile([128, NQB], f32, tag="rs")
            iv = cst.tile([128, NQB], f32, tag="iv")
            ivb = cst.tile([128, NQB], bf16, tag="ivb")
            twT = cst.tile([128, NKC], bf16, tag="twT")
            wt = cst.tile([128, 64], bf16, tag="wt")
            cbf = cst.tile([128, 2], f32, tag="cbf")
            usb = cst.tile([128, 2], f32, tag="usb")

            twp = ptw.tile([128, NKC], f32, tag="twp")
            # u accumulators: allocated lazily (first emit_u) out of the qa
            # psum rotation, one PER d-half.  A start=True matmul lazily
            # zeroes its whole 2KB psum bank, so each concurrently-open
            # accumulation group needs its own bank.
            up = {}

            qa = (qa0, qa1)

            def mask_for(i):
                if i == 0:
                    return mxn[:, 0:384]
                if i == NQB - 1:
                    return mxn[:, 768:1152]
                return mxn[:, 384:768]

            def xn_chunk(c, dh):
                o = 1152 + c * 256 + dh * 128
                return mxn[:, o:o + 128]

            # Pool: memset warmup tile, then PE keeps itself busy ramping.
            # Warmup psum shares the score-pair pool (all 8 banks are spoken
            # for); its WAR hazard resolves before the first real scores.
            nc.gpsimd.memset(wt[:], 0.0)
            wu = ppr.tile([128, 768], f32, tag="ppr", name="wu")
            for _ in range(44):
                nc.tensor.matmul(wu[0:64, 0:64], wt[:, 0:64], wt[:, 0:64],
                                 start=True, stop=True)

            # DMAs: xt pieces feed qa sub-chunks and early score pairs;
            # masks before the first STT; xn halves before u needs them.
            xtd_v = xt_d.rearrange("(t p) k -> p t k", p=128)
            nc.sync.dma_start(xtv[:, :, 0:640], xtd_v[:, :, 0:640])
            nc.sync.dma_start(apk[:], ap_d[:])
            nc.gpsimd.tensor_copy(cbf[:], apk[:, 512:514])
            nc.sync.dma_start(xtv[:, :, 640:1280], xtd_v[:, :, 640:1280])
            nc.sync.dma_start(xtv[:, :, 1280:1920], xtd_v[:, :, 1280:1920])
            nc.sync.dma_start(mxn[:, 0:1152], mxn_d[:, 0:1152])
            nc.sync.dma_start(xtv[:, :, 1920:NK], xtd_v[:, :, 1920:NK])
            nc.sync.dma_start(mxn[:, 1152:3456], mxn_d[:, 1152:3456])
            nc.sync.dma_start(mxn[:, 3456:MXW], mxn_d[:, 3456:MXW])

            # qa projection: sub-chunk s = queries [512s, 512(s+1)), i.e. xt
            # cols [128+512s, 640+512s).  The bias rides the psum->sbuf copy.
            qa_ps = {}

            def qa_mm(s_, m):
                c0 = HALO + 512 * s_
                ps = pqa.tile([128, 512], f32, tag="pqa",
                              name=f"pqa_{s_}_{m}")
                for k in range(2):
                    nc.tensor.matmul(
                        ps[:],
                        apk[:, k * 256 + m * 128:k * 256 + (m + 1) * 128],
                        xtv[:, k, c0:c0 + 512],
                        start=(k == 0), stop=(k == 1),
                    )
                qa_ps[(s_, m)] = ps

            def qa_copy(s_, m, eng):
                ps = qa_ps.pop((s_, m))
                dst = qa[m][:, 512 * s_:512 * (s_ + 1)]
                if eng == "dve":
                    nc.vector.tensor_scalar_add(dst, ps[:], cbf[:, m:m + 1])
                else:
                    nc.scalar.activation(dst, ps[:], AF.Identity,
                                         bias=cbf[:, m:m + 1])

            for s_ in (0, 1):
                qa_mm(s_, 0)
                qa_mm(s_, 1)
            qa_copy(0, 0, "dve")
            qa_copy(0, 1, "act")
            qa_copy(1, 0, "dve")

            em_tiles = {}

            def emit_tw(jc):
                blocks = [i for i in range(jc - 2, jc + 1) if 0 <= i < NQB]
                for i in blocks:
                    et, off = em_tiles[i // 2], (i % 2) * 384
                    nc.tensor.matmul(
                        twp[:, jc:jc + 1],
                        et[:, off + (jc - i) * 128:off + (jc - i + 1) * 128],
                        ivb[:, i:i + 1],
                        start=(i == blocks[0]), stop=(i == blocks[-1]),
                    )

            def emit_u(jc):
                for dh in range(2):
                    if dh not in up:
                        up[dh] = pqa.tile([128, 512], f32, tag="pqa",
                                          name=f"pu{dh}")
                    nc.tensor.matmul(
                        up[dh][:, 0:1],
                        xn_chunk(jc, dh),
                        twT[:, jc:jc + 1],
                        start=(jc == 0), stop=(jc == NKC - 1),
                    )

            # Streamed attention over 8 block pairs + drain stages.  Each
            # consumer runs >=1 pair behind its producers so every
            # cross-engine dependency has slack.
            QA_MM2 = {0: ((2, 0),), 1: ((2, 1), (3, 0)), 2: ((3, 1),)}
            QA_DEFER = {0: (((1, 1), "act"), ((2, 0), "dve")),
                        1: (((2, 1), "act"), ((3, 0), "dve")),
                        2: (((3, 1), "dve"),)}
            UPLAN = {4: (0, 1, 2), 5: (3, 4, 5), 6: (6, 7, 8),
                     7: (9, 10, 11), 10: tuple(range(12, 18))}
            for p in range(12):
                i0, i1 = 2 * p, 2 * p + 1
                if p < NPAIR:
                    # odd block sits at column 512 so neither block's 384-col
                    # matmul output crosses a 2KB PSUM bank boundary (psum
                    # accumulation breaks across banks)
                    psc = ppr.tile([128, 1024], f32, tag="ppr",
                                   name=f"psc{p}")
                    for bi, off in ((i0, 0), (i1, 512)):
                        for k in range(2):
                            nc.tensor.matmul(
                                psc[:, off:off + 384],
                                qa[k][:, bi * 128:(bi + 1) * 128],
                                xtv[:, k, 128 * bi:128 * bi + 384],
                                start=(k == 0), stop=(k == 1),
                            )
                for (s2_, m2_) in QA_MM2.get(p, ()):
                    qa_mm(s2_, m2_)
                if 2 <= p < 9:
                    for jc in ((2 * (p - 2), 2 * (p - 2) + 1) if p < 8
                               else range(12, 18)):
                        emit_tw(jc)
                for jc in UPLAN.get(p, ()):
                    emit_u(jc)
                if p < NPAIR:
                    ex = wrk.tile([128, 768], bf16, tag="ex", bufs=2,
                                  name=f"ex{p}")
                    psc_v = psc.rearrange("p (g c) -> p g c", g=2)
                    ex_v = ex.rearrange("p (g c) -> p g c", g=2)
                    nc.scalar.activation(ex_v[:, :, :], psc_v[:, :, 0:384],
                                         AF.Exp)
                    em = wrk.tile([128, 768], bf16, tag="em", bufs=3,
                                  name=f"em{p}")
                    em_tiles[p] = em
                    for bi, off in ((i0, 0), (i1, 384)):
                        nc.vector.scalar_tensor_tensor(
                            em[:, off:off + 384], ex[:, off:off + 384], 1.0,
                            mask_for(bi), ALU.mult, ALU.mult,
                            accum_out=rs[:, bi:bi + 1],
                        )
                    nc.vector.reciprocal(iv[:, i0:i1 + 1], rs[:, i0:i1 + 1])
                    nc.gpsimd.tensor_copy(ivb[:, i0:i1 + 1], iv[:, i0:i1 + 1])
                for (sm, eng) in QA_DEFER.get(p, ()):
                    qa_copy(sm[0], sm[1], eng)
                if 3 <= p < 10:
                    if p < 9:
                        g0 = 2 * (p - 3)
                        nc.vector.tensor_copy(twT[:, g0:g0 + 2],
                                              twp[:, g0:g0 + 2])
                    else:
                        nc.vector.tensor_copy(twT[:, 12:18], twp[:, 12:18])

            nc.scalar.copy(usb[:, 0:1], up[0][:, 0:1])
            nc.scalar.copy(usb[:, 1:2], up[1][:, 0:1])
            nc.sync.dma_start(u_d[:], usb[:])
            if DEBUG_DUMP:
                nc.sync.dma_start(dqa0_d[:], qa0[:])
                nc.sync.dma_start(dqa1_d[:], qa1[:])
                nc.sync.dma_start(drs_d[:], rs[:])
                nc.sync.dma_start(dtw_d[:], twT[:])

    nc.compile()
    return nc


def _numpy_fallback(x, Wq, bq, Wk, bk, Wv, bv, window_size):
    out = np.zeros((B, H), np.float64)
    xs = x.astype(np.float64)
    A = (Wq.astype(np.float64) @ Wk.astype(np.float64).T) / np.sqrt(H)
    cb = (Wk.astype(np.float64) @ bq.astype(np.float64)) / np.sqrt(H)
    idx = np.arange(x.shape[1])
    band = np.abs(idx[:, None] - idx[None, :]) <= int(window_size)
    for b in range(x.shape[0]):
        qa = xs[b] @ A + cb
        sc = qa @ xs[b].T
        e = np.exp(sc - sc.max(axis=-1, keepdims=True)) * band
        w = e / e.sum(-1, keepdims=True)
        tw = w.sum(axis=0)
        out[b] = (tw @ xs[b] / x.shape[1]) @ Wv.astype(np.float64) + bv
    return out.astype(np.float32)


def kernel(x, Wq, bq, Wk, bk, Wv, bv, window_size):
    x = np.asarray(x)
    Wq, bq = np.asarray(Wq), np.asarray(bq)
    Wk, bk = np.asarray(Wk), np.asarray(bk)
    Wv, bv = np.asarray(Wv), np.asarray(bv)
    if int(window_size) != W or x.shape != (B, S, H):
        return _numpy_fallback(x, Wq, bq, Wk, bk, Wv, bv, window_size)

    from concourse.bass_utils import run_bass_kernel_spmd

    if "nc" not in _CACHE:
        _CACHE["nc"] = _build()
    nc = _CACHE["nc"]

    A64 = (Wq.astype(np.float64) @ Wk.astype(np.float64).T) / np.sqrt(H)
    cb64 = (Wk.astype(np.float64) @ bq.astype(np.float64)) / np.sqrt(H)

    # apk: A packed as [p, k*256 + m*128 + j] = A[k*128+p, m*128+j],
    # cb halves in cols 512/513.
    ap_np = np.zeros((128, 516), np.float32)
    for k in range(2):
        for m in range(2):
            ap_np[:, k * 256 + m * 128:k * 256 + (m + 1) * 128] = \
                A64[k * 128:(k + 1) * 128, m * 128:(m + 1) * 128]
    ap_np[:, 512] = cb64[0:128]
    ap_np[:, 513] = cb64[128:256]
    ap_np = ap_np.astype(BF16)

    r = np.arange(128)[:, None]
    c = np.arange(384)[None, :]
    band = (np.abs(c - r - HALO) <= W)
    mkM = band.astype(np.float32)
    mkA0 = (band & (c >= 128)).astype(np.float32)   # h=0 block 0
    mkB1 = (band & (c < 256)).astype(np.float32)    # h=1 block 15

    in_maps = []
    for core in range(8):
        b, h = core // 2, core % 2
        q0 = h * SH
        xpad = np.zeros((NK, H), np.float32)
        lo, hi = q0 - HALO, q0 + SH + HALO
        slo, shi = max(lo, 0), min(hi, S)
        xpad[slo - lo: shi - lo, :] = x[b, slo:shi, :]

        xt_np = np.ascontiguousarray(xpad.T).astype(BF16)  # [256, NK]

        mxn_np = np.zeros((128, MXW), np.float32)
        mxn_np[:, 0:384] = mkA0 if h == 0 else mkM
        mxn_np[:, 384:768] = mkM
        mxn_np[:, 768:1152] = mkB1 if h == 1 else mkM
        xnv = xpad.reshape(NKC, 128, 256).transpose(1, 0, 2).reshape(128, -1)
        mxn_np[:, 1152:] = xnv
        in_maps.append({
            "xt": xt_np, "apk": ap_np, "mxn": mxn_np.astype(BF16),
        })

    import os
    trace = bool(os.environ.get("BASS_TRACE"))
    res = run_bass_kernel_spmd(nc, in_maps, list(range(8)), trace=trace)
    _CACHE["last"] = res

    out = np.zeros((B, H), np.float64)
    for b in range(B):
        u = np.zeros(H, np.float64)
        for h in range(2):
            uc = res.results[2 * b + h]["u"].astype(np.float64)
            u += np.concatenate([uc[:, 0], uc[:, 1]])
        out[b] = (u / S) @ Wv.astype(np.float64) + bv
    return out.astype(np.float32)
